# revision 1
# baseline (speedup 1.0000x reference)
"""Trainium2 Bass kernel for nn_DistDistance (retrieval_knn).

Reference computation (per batch b):
    dist2[l2, l1] = || c1[b,l1,:] - c2[b,l2,:] + eps ||^2
                  = s1(l1) + s2(l2) - 2*c1.c2 + 2*eps*(sum1(l1) - sum2(l2)) + D*eps^2
    out[b] = mean_l2( sqrt(max(min_l1 dist2, 0)) * resolution )

Device strategy (8 cores, SPMD):
  - core c handles batch b=c//2, query half h=c%2 (2048 of the 4096 l2 points).
  - dist2 is split into an l1-dependent bracket and an l2-dependent constant:
        bracket(l2,l1) = s1(l1) + 2*eps*sum1(l1) - 2*c1.c2   (computed on device)
        t2(l2) = s2(l2) - 2*eps*sum2(l2) + D*eps^2           (added on host)
    min_l1 dist2 = min_l1 bracket + t2, and sqrt is monotone, so the device
    only needs min_l1 bracket per l2 point.
  - bracket is a single K=15 matmul via augmented coordinates: each fp32
    value is split into hi+lo bf16 parts so that all 4 cross products are
    present (exact to ~2^-18 relative), and the s1-term is a 3-way bf16
    split. The PE runs bf16 at 1 cycle/row vs fp32's 4.
  - The 15-row operands are replicated at SBUF base partitions 0/32/64/96 so
    4 matmuls run concurrently in distinct PE row-groups (tile_position),
    each targeting its own PSUM bank.
  - DVE reduce-min over l1 (free axis) per 128-query tile; results [128, 16]
    are DMA'd out; host adds t2, takes sqrt and the mean.
"""

import numpy as np
import ml_dtypes
from contextlib import ExitStack

import concourse.mybir as mybir
import concourse.tile as tile
from concourse import bacc, bass_utils

B, L1, L2, D = 4, 4096, 4096, 3
EPS = 1e-6
NCORES = 8
L2H = L2 // 2      # l2 points per core
NT = L2H // 128    # l2 tiles per core
K = 15             # augmented contraction rows
CHUNK = 512        # l1 chunk = one PSUM bank (fp32)
NROUND = L1 // (4 * CHUNK)  # matmul rounds per l2 tile (4 packed chunks each)

BF16 = ml_dtypes.bfloat16


def _build_nc():
    nc = bacc.Bacc("TRN2", num_devices=NCORES)
    vt = nc.dram_tensor("vt", [128, L2H], mybir.dt.bfloat16, kind="ExternalInput").ap()
    ut = nc.dram_tensor(
        "ut", [128, NROUND * CHUNK], mybir.dt.bfloat16, kind="ExternalInput"
    ).ap()
    out = nc.dram_tensor("out", [128, NT], mybir.dt.float32, kind="ExternalOutput").ap()

    with tile.TileContext(nc) as tc, ExitStack() as ctx:
        const = ctx.enter_context(tc.tile_pool(name="const", bufs=1))
        psum = ctx.enter_context(tc.tile_pool(name="psum", bufs=2, space="PSUM"))
        stats = ctx.enter_context(tc.tile_pool(name="stats", bufs=1))
        small = ctx.enter_context(tc.tile_pool(name="small", bufs=2))

        vt_sb = const.tile([128, L2H], mybir.dt.bfloat16)
        ut_sb = const.tile([128, NROUND * CHUNK], mybir.dt.bfloat16)
        nc.sync.dma_start(out=vt_sb, in_=vt)
        nc.sync.dma_start(out=ut_sb, in_=ut)

        mins = stats.tile([128, NT], mybir.dt.float32)

        for t in range(NT):
            ms = []
            for r in range(NROUND):
                ps = psum.tile([128, 4 * CHUNK], mybir.dt.float32, tag="ps")
                for j in range(4):
                    bp = 32 * j
                    nc.tensor.matmul(
                        ps[:, j * CHUNK : (j + 1) * CHUNK],
                        lhsT=vt_sb[bp : bp + K, t * 128 : (t + 1) * 128],
                        rhs=ut_sb[bp : bp + K, r * CHUNK : (r + 1) * CHUNK],
                        start=True,
                        stop=True,
                        tile_position=(bp, 0),
                    )
                m = small.tile([128, 1], mybir.dt.float32, tag=f"m{r}")
                nc.vector.tensor_reduce(
                    m, ps, axis=mybir.AxisListType.X, op=mybir.AluOpType.min
                )
                ms.append(m)
            nc.vector.tensor_tensor(
                mins[:, t : t + 1], ms[0], ms[1], op=mybir.AluOpType.min
            )

        nc.sync.dma_start(out=out, in_=mins)

    nc.finalize()
    return nc


def _split2(x):
    hi = x.astype(BF16)
    lo = (x - hi.astype(np.float32)).astype(BF16)
    return hi, lo


def _split3(x):
    p = x.astype(BF16)
    r1 = x - p.astype(np.float32)
    q = r1.astype(BF16)
    r = (r1 - q.astype(np.float32)).astype(BF16)
    return p, q, r


def _prep_core(c1b, c2b, h):
    """Build the vt/ut operands for core (batch data c1b/c2b, l2 half h).

    Returns (vt [128, L2H] bf16, ut [128, NROUND*CHUNK] bf16, t2 [L2H] f32).
    """
    c2h = c2b[h * L2H : (h + 1) * L2H]  # [L2H, 3]

    # U side (contour1, streamed): rows pair as
    #   (h1,hw)x3, (l1,hw)x3, (h1,lw)x3, (l1,lw)x3, (a1 parts, 1)x3
    s1 = np.sum(c1b * c1b, axis=-1, dtype=np.float32)
    sum1 = np.sum(c1b, axis=-1, dtype=np.float32)
    a1 = s1 + 2.0 * EPS * sum1
    h1, l1 = _split2(c1b.T)  # [3, L1] each
    p, q, r = _split3(a1[None, :])  # [1, L1] each
    urows = np.concatenate([h1, l1, h1, l1, p, q, r], axis=0)  # [15, L1]

    w = -2.0 * c2h.T  # [3, L2H]
    hw, lw = _split2(w)
    ones = np.ones((3, L2H), dtype=BF16)
    vrows = np.concatenate([hw, hw, lw, lw, ones], axis=0)  # [15, L2H]

    vt = np.zeros((128, L2H), dtype=BF16)
    ut = np.zeros((128, NROUND * CHUNK), dtype=BF16)
    for j in range(4):
        vt[32 * j : 32 * j + K, :] = vrows
        for r_ in range(NROUND):
            c = 4 * r_ + j
            ut[32 * j : 32 * j + K, r_ * CHUNK : (r_ + 1) * CHUNK] = urows[
                :, c * CHUNK : (c + 1) * CHUNK
            ]

    s2 = np.sum(c2h * c2h, axis=-1, dtype=np.float32)
    sum2 = np.sum(c2h, axis=-1, dtype=np.float32)
    t2 = s2 - 2.0 * EPS * sum2 + D * EPS * EPS
    return vt, ut, t2


_NC_CACHE = []


def _get_nc():
    if not _NC_CACHE:
        _NC_CACHE.append(_build_nc())
    return _NC_CACHE[0]


def kernel(contour1, contour2, resolution):
    c1 = np.asarray(contour1, dtype=np.float32)
    c2 = np.asarray(contour2, dtype=np.float32)
    res = float(np.asarray(resolution).reshape(-1)[0])

    in_maps = []
    t2s = []
    for core in range(NCORES):
        b, h = core // 2, core % 2
        vt, ut, t2 = _prep_core(c1[b], c2[b], h)
        in_maps.append({"vt": vt, "ut": ut})
        t2s.append(t2)

    nc = _get_nc()
    results = bass_utils.run_bass_kernel_spmd(
        nc, in_maps, core_ids=list(range(NCORES))
    ).results

    out = np.empty((B,), dtype=np.float32)
    for b in range(B):
        halves = []
        for h in range(2):
            r = results[2 * b + h]["out"]  # [128, NT]; [p, t] -> l2 = t*128+p
            partial = r.T.reshape(L2H)
            d2 = partial + t2s[2 * b + h]
            halves.append(np.sqrt(np.maximum(d2, 0.0)))
        min_dist = np.concatenate(halves)
        out[b] = np.float32(np.mean(min_dist * res))
    return out


# revision 2
# speedup vs baseline: 1.0470x; 1.0470x over previous
"""Trainium2 Bass kernel for nn_DistDistance (retrieval_knn).

Reference computation (per batch b):
    dist2[l2, l1] = || c1[b,l1,:] - c2[b,l2,:] + eps ||^2
                  = s1(l1) + s2(l2) - 2*c1.c2 + 2*eps*(sum1(l1) - sum2(l2)) + D*eps^2
    out[b] = mean_l2( sqrt(max(min_l1 dist2, 0)) * resolution )

Device strategy (8 cores, SPMD):
  - core c handles batch b=c//2, query half h=c%2 (2048 of the 4096 l2 points).
  - dist2 splits into an l1-dependent bracket and an l2-dependent constant:
        bracket(l2,l1) = s1(l1) + 2*eps*sum1(l1) - 2*c1.c2   (PE matmul)
        t2(l2) = s2(l2) - 2*eps*sum2(l2) + D*eps^2           (tiny, host-computed)
    and min_l1 dist2 = min_l1 bracket + t2 (sqrt is monotone), so the device
    computes min_l1 per query point; host does sqrt and the mean.
  - bracket is one K=15 matmul via augmented coordinates: each fp32 value is
    split hi+lo into bf16 so all 4 cross products are present (~2^-18 exact),
    and the s1-term is a 3-way bf16 split. bf16 streams 1 PE cycle/row vs
    fp32's 4.
  - The 15-row operands are replicated at SBUF base partitions 0/32/64/96 so
    4 matmuls run concurrently in distinct PE row-groups (tile_position),
    each into its own PSUM bank.
  - min over l1 (free axis) is the bottleneck; it is split across two engines:
      (a) some query tiles: DVE reduce-min fp32 straight from PSUM;
      (b) the rest: ScalarE converts PSUM->SBUF fp16 while adding the
          per-partition t2 bias (values become dist2-sized, so fp16 is
          accurate), then DVE runs a 2x-mode fp16 tensor_tensor min
          tournament, which is ~1.7x cheaper per element for the DVE.
    ScalarE and VectorE PSUM reads proceed in parallel on different banks.
"""

import numpy as np
import ml_dtypes
from contextlib import ExitStack

import concourse.mybir as mybir
import concourse.tile as tile
from concourse import bacc, bass_utils

B, L1, L2, D = 4, 4096, 4096, 3
EPS = 1e-6
NCORES = 8
L2H = L2 // 2      # l2 points per core
NT = L2H // 128    # l2 tiles per core
K = 15             # augmented contraction rows
CHUNK = 512        # l1 chunk = one PSUM bank (fp32)
NROUND = L1 // (4 * CHUNK)  # matmul rounds per l2 tile (4 packed chunks each)
HALF = L1 // 2     # l1 elements per PSUM tile

# l2-tiles reduced straight from PSUM by the DVE; the rest go through the
# ScalarE fp16 conversion path. Ratio balances DVE and ScalarE totals.
A_TILES = frozenset({0, 5, 11})

BF16 = ml_dtypes.bfloat16


def _build_nc():
    nc = bacc.Bacc("TRN2", num_devices=NCORES)
    vt = nc.dram_tensor("vt", [128, L2H], mybir.dt.bfloat16, kind="ExternalInput").ap()
    ut = nc.dram_tensor(
        "ut", [128, NROUND * CHUNK], mybir.dt.bfloat16, kind="ExternalInput"
    ).ap()
    t2c = nc.dram_tensor("t2c", [128, NT], mybir.dt.float32, kind="ExternalInput").ap()
    out = nc.dram_tensor("out", [128, NT], mybir.dt.float32, kind="ExternalOutput").ap()

    with tile.TileContext(nc) as tc, ExitStack() as ctx:
        const = ctx.enter_context(tc.tile_pool(name="const", bufs=1))
        psum = ctx.enter_context(tc.tile_pool(name="psum", bufs=2, space="PSUM"))
        fp16p = ctx.enter_context(tc.tile_pool(name="fp16p", bufs=2))
        stats = ctx.enter_context(tc.tile_pool(name="stats", bufs=1))
        small = ctx.enter_context(tc.tile_pool(name="small", bufs=2))

        vt_sb = const.tile([128, L2H], mybir.dt.bfloat16)
        ut_sb = const.tile([128, NROUND * CHUNK], mybir.dt.bfloat16)
        t2_sb = const.tile([128, NT], mybir.dt.float32)
        nc.sync.dma_start(out=vt_sb, in_=vt)
        nc.sync.dma_start(out=ut_sb, in_=ut)
        nc.sync.dma_start(out=t2_sb, in_=t2c)

        mins = stats.tile([128, NT], mybir.dt.float32)

        for t in range(NT):
            pss = []
            for r in range(NROUND):
                ps = psum.tile([128, 4 * CHUNK], mybir.dt.float32, tag="ps")
                for j in range(4):
                    bp = 32 * j
                    nc.tensor.matmul(
                        ps[:, j * CHUNK : (j + 1) * CHUNK],
                        lhsT=vt_sb[bp : bp + K, t * 128 : (t + 1) * 128],
                        rhs=ut_sb[bp : bp + K, r * CHUNK : (r + 1) * CHUNK],
                        start=True,
                        stop=True,
                        tile_position=(bp, 0),
                    )
                pss.append(ps)

            t2col = t2_sb[:, t : t + 1]
            if t in A_TILES:
                # (a) DVE reduces fp32 straight from PSUM; t2 added at the end.
                ms = []
                for r in range(NROUND):
                    m = small.tile([128, 1], mybir.dt.float32, tag=f"m{r}")
                    nc.vector.tensor_reduce(
                        m, pss[r], axis=mybir.AxisListType.X, op=mybir.AluOpType.min
                    )
                    ms.append(m)
                mb = small.tile([128, 1], mybir.dt.float32, tag="mb")
                nc.vector.tensor_tensor(mb, ms[0], ms[1], op=mybir.AluOpType.min)
                nc.vector.tensor_tensor(
                    mins[:, t : t + 1], mb, t2col, op=mybir.AluOpType.add
                )
            else:
                # (b) ScalarE: dist2 = psum + t2 (per-partition bias), cast to
                # fp16 in SBUF; DVE: 2x-mode fp16 min tournament.
                f0 = fp16p.tile([128, HALF], mybir.dt.float16, tag="f0")
                f1 = fp16p.tile([128, HALF], mybir.dt.float16, tag="f1")
                nc.scalar.add(f0, pss[0], add=t2col)
                nc.scalar.add(f1, pss[1], add=t2col)
                nc.vector.tensor_tensor(f0, f0, f1, op=mybir.AluOpType.min)
                w = HALF // 2
                while w >= 16:
                    nc.vector.tensor_tensor(
                        f0[:, 0:w], f0[:, 0:w], f0[:, w : 2 * w], op=mybir.AluOpType.min
                    )
                    w //= 2
                nc.vector.tensor_reduce(
                    mins[:, t : t + 1],
                    f0[:, 0 : 2 * w],
                    axis=mybir.AxisListType.X,
                    op=mybir.AluOpType.min,
                )

        nc.sync.dma_start(out=out, in_=mins)

    nc.finalize()
    return nc


def _split2(x):
    hi = x.astype(BF16)
    lo = (x - hi.astype(np.float32)).astype(BF16)
    return hi, lo


def _split3(x):
    p = x.astype(BF16)
    r1 = x - p.astype(np.float32)
    q = r1.astype(BF16)
    r = (r1 - q.astype(np.float32)).astype(BF16)
    return p, q, r


def _prep_core(c1b, c2b, h):
    """Build vt/ut/t2c operands for one core (batch data c1b/c2b, l2 half h)."""
    c2h = c2b[h * L2H : (h + 1) * L2H]  # [L2H, 3]

    # U side (contour1, streamed): rows pair as
    #   (h1,hw)x3, (l1,hw)x3, (h1,lw)x3, (l1,lw)x3, (a1 parts, 1)x3
    s1 = np.sum(c1b * c1b, axis=-1, dtype=np.float32)
    sum1 = np.sum(c1b, axis=-1, dtype=np.float32)
    a1 = s1 + 2.0 * EPS * sum1
    h1, l1 = _split2(c1b.T)  # [3, L1] each
    p, q, r = _split3(a1[None, :])  # [1, L1] each
    urows = np.concatenate([h1, l1, h1, l1, p, q, r], axis=0)  # [15, L1]

    w = -2.0 * c2h.T  # [3, L2H]
    hw, lw = _split2(w)
    ones = np.ones((3, L2H), dtype=BF16)
    vrows = np.concatenate([hw, hw, lw, lw, ones], axis=0)  # [15, L2H]

    vt = np.zeros((128, L2H), dtype=BF16)
    ut = np.zeros((128, NROUND * CHUNK), dtype=BF16)
    for j in range(4):
        vt[32 * j : 32 * j + K, :] = vrows
        for r_ in range(NROUND):
            c = 4 * r_ + j
            ut[32 * j : 32 * j + K, r_ * CHUNK : (r_ + 1) * CHUNK] = urows[
                :, c * CHUNK : (c + 1) * CHUNK
            ]

    s2 = np.sum(c2h * c2h, axis=-1, dtype=np.float32)
    sum2 = np.sum(c2h, axis=-1, dtype=np.float32)
    t2 = (s2 - 2.0 * EPS * sum2 + D * EPS * EPS).astype(np.float32)
    t2c = t2.reshape(NT, 128).T.copy()  # [128, NT]; [p, t] -> l2 = t*128+p
    return vt, ut, t2c


_NC_CACHE = []


def _get_nc():
    if not _NC_CACHE:
        _NC_CACHE.append(_build_nc())
    return _NC_CACHE[0]


def kernel(contour1, contour2, resolution):
    c1 = np.asarray(contour1, dtype=np.float32)
    c2 = np.asarray(contour2, dtype=np.float32)
    res = float(np.asarray(resolution).reshape(-1)[0])

    in_maps = []
    for core in range(NCORES):
        b, h = core // 2, core % 2
        vt, ut, t2c = _prep_core(c1[b], c2[b], h)
        in_maps.append({"vt": vt, "ut": ut, "t2c": t2c})

    nc = _get_nc()
    results = bass_utils.run_bass_kernel_spmd(
        nc, in_maps, core_ids=list(range(NCORES))
    ).results

    out = np.empty((B,), dtype=np.float32)
    for b in range(B):
        halves = []
        for h in range(2):
            r = results[2 * b + h]["out"]  # [128, NT]; [p, t] -> l2 = t*128+p
            d2 = r.T.reshape(L2H)  # min dist2 per l2 point (t2 already added)
            halves.append(np.sqrt(np.maximum(d2, 0.0)))
        min_dist = np.concatenate(halves)
        out[b] = np.float32(np.mean(min_dist * res))
    return out


# revision 10
# speedup vs baseline: 1.1751x; 1.1223x over previous
"""Trainium2 Bass kernel for nn_DistDistance (retrieval_knn).

Reference computation (per batch b):
    dist2[l2, l1] = || c1[b,l1,:] - c2[b,l2,:] + eps ||^2
                  = s1(l1) + s2(l2) - 2*c1.c2 + 2*eps*(sum1(l1) - sum2(l2)) + D*eps^2
    out[b] = mean_l2( sqrt(max(min_l1 dist2, 0)) * resolution )

Device strategy (8 cores, SPMD):
  - core c handles batch b=c//2, query half h=c%2 (2048 of the 4096 l2 points).
  - dist2 splits into an l1-dependent bracket and an l2-dependent constant:
        bracket(l2,l1) = s1(l1) + 2*eps*sum1(l1) - 2*c1.c2   (PE matmul)
        t2(l2) = s2(l2) - 2*eps*sum2(l2) + D*eps^2           (tiny, host-computed)
    and min_l1 dist2 = min_l1 bracket + t2 (sqrt is monotone), so the device
    computes a min per query point; host does sqrt and the mean.
  - bracket is one K=15 matmul via augmented coordinates: each fp32 value is
    split hi+lo into bf16 so all 4 cross products are present (~2^-18 exact),
    and the s1-term is a 3-way bf16 split. bf16 streams 1 PE cycle/row vs
    fp32's 4.
  - The 15-row operands are replicated at SBUF base partitions 0/32/64/96 so
    4 matmuls run concurrently in distinct PE row-groups (tile_position),
    each into its own PSUM bank.
  - min over l1 (free axis) is the bottleneck. Every PSUM slab [128, 2048] is
    split column-wise across both PSUM-capable engines, which read different
    banks in parallel:
      * ScalarE: cols [0:1792] -> dist2 = psum + t2 (per-partition bias AP),
        cast fp16 into SBUF (values are dist2-sized, so fp16 is accurate);
      * VectorE: cols [1792:2048] reduce-min fp32 straight from PSUM.
    VectorE then runs a 2x-mode fp16 tensor_tensor min tournament over the
    converted columns. The 1792/256 split balances ScalarE and VectorE.
"""

import numpy as np
import ml_dtypes
from contextlib import ExitStack

import concourse.mybir as mybir
import concourse.tile as tile
from concourse import bacc, bass_utils

B, L1, L2, D = 4, 4096, 4096, 3
EPS = 1e-6
NCORES = 8
L2H = L2 // 2      # l2 points per core
NT = L2H // 128    # l2 tiles per core
K = 15             # augmented contraction rows
CHUNK = 512        # l1 chunk = one PSUM bank (fp32)
NROUND = L1 // (4 * CHUNK)  # matmul rounds per l2 tile (4 packed chunks each)
HALF = L1 // 2     # l1 elements per PSUM slab
XACT = 1664        # slab columns converted by ScalarE (rest go DVE-direct)
XDVE = HALF - XACT

BF16 = ml_dtypes.bfloat16


def _build_nc():
    nc = bacc.Bacc("TRN2", num_devices=NCORES)
    vt = nc.dram_tensor("vt", [128, L2H], mybir.dt.bfloat16, kind="ExternalInput").ap()
    ut = nc.dram_tensor(
        "ut", [128, NROUND * CHUNK], mybir.dt.bfloat16, kind="ExternalInput"
    ).ap()
    t2c = nc.dram_tensor("t2c", [128, NT], mybir.dt.float32, kind="ExternalInput").ap()
    out = nc.dram_tensor("out", [128, NT], mybir.dt.float32, kind="ExternalOutput").ap()

    with tile.TileContext(nc) as tc, ExitStack() as ctx:
        const = ctx.enter_context(tc.tile_pool(name="const", bufs=1))
        psum = ctx.enter_context(tc.tile_pool(name="psum", bufs=2, space="PSUM"))
        fp16p = ctx.enter_context(tc.tile_pool(name="fp16p", bufs=4))
        stats = ctx.enter_context(tc.tile_pool(name="stats", bufs=1))
        small = ctx.enter_context(tc.tile_pool(name="small", bufs=2))

        t2_sb = const.tile([128, NT], mybir.dt.float32)
        ut_sb = const.tile([128, NROUND * CHUNK], mybir.dt.bfloat16)
        vt_sb = const.tile([128, L2H], mybir.dt.bfloat16)
        # DMA order = dependency order (ut + first vt block gate tile 0), and
        # the vt blocks go on a different DGE queue so they overlap the ut
        # transfer instead of queueing behind it.
        nc.scalar.dma_start(out=t2_sb, in_=t2c)
        for r_ in range(NROUND):
            nc.sync.dma_start(
                out=ut_sb[:, r_ * CHUNK : (r_ + 1) * CHUNK],
                in_=ut[:, r_ * CHUNK : (r_ + 1) * CHUNK],
            )
        VBLK = 512
        for blk in range(L2H // VBLK):
            nc.gpsimd.dma_start(
                out=vt_sb[:, blk * VBLK : (blk + 1) * VBLK],
                in_=vt[:, blk * VBLK : (blk + 1) * VBLK],
            )

        mins = stats.tile([128, NT], mybir.dt.float32)
        # Touch ScalarE's Identity early so the ACT table load overlaps DMA.
        warm = small.tile([128, 1], mybir.dt.float32, tag="warm")
        nc.scalar.add(warm, t2_sb[:, 0:1], add=t2_sb[:, 0:1])

        for t in range(NT):
            pss = []
            for r in range(NROUND):
                ps = psum.tile([128, 4 * CHUNK], mybir.dt.float32, tag="ps")
                for j in range(4):
                    bp = 32 * j
                    nc.tensor.matmul(
                        ps[:, j * CHUNK : (j + 1) * CHUNK],
                        lhsT=vt_sb[bp : bp + K, t * 128 : (t + 1) * 128],
                        rhs=ut_sb[bp : bp + K, r * CHUNK : (r + 1) * CHUNK],
                        start=True,
                        stop=True,
                        tile_position=(bp, 0),
                    )
                pss.append(ps)

            t2col = t2_sb[:, t : t + 1]
            # ScalarE: dist2 = psum + t2, cast fp16 into SBUF (first XACT cols).
            f0 = fp16p.tile([128, XACT], mybir.dt.float16, tag="f0")
            f1 = fp16p.tile([128, XACT], mybir.dt.float16, tag="f1")
            nc.scalar.add(f0, pss[0][:, 0:XACT], add=t2col)
            nc.scalar.add(f1, pss[1][:, 0:XACT], add=t2col)

            # DVE: fp32 reduce-min of the leftover columns straight from PSUM.
            d0 = small.tile([128, 1], mybir.dt.float32, tag="d0")
            d1 = small.tile([128, 1], mybir.dt.float32, tag="d1")
            nc.vector.tensor_reduce(
                d0, pss[0][:, XACT:HALF], axis=mybir.AxisListType.X,
                op=mybir.AluOpType.min,
            )
            nc.vector.tensor_reduce(
                d1, pss[1][:, XACT:HALF], axis=mybir.AxisListType.X,
                op=mybir.AluOpType.min,
            )
            dm = small.tile([128, 1], mybir.dt.float32, tag="dm")
            nc.vector.tensor_tensor(dm, d0, d1, op=mybir.AluOpType.min)
            nc.vector.tensor_tensor(dm, dm, t2col, op=mybir.AluOpType.add)

            # DVE: 2x-mode fp16 min tournament over the converted columns.
            nc.vector.tensor_tensor(f0, f0, f1, op=mybir.AluOpType.min)
            w = XACT // 2
            while w >= 54:
                nc.vector.tensor_tensor(
                    f0[:, 0:w], f0[:, 0:w], f0[:, w : 2 * w], op=mybir.AluOpType.min
                )
                w //= 2
            fm = small.tile([128, 1], mybir.dt.float32, tag="fm")
            nc.vector.tensor_reduce(
                fm, f0[:, 0 : 2 * w], axis=mybir.AxisListType.X,
                op=mybir.AluOpType.min,
            )
            nc.vector.tensor_tensor(
                mins[:, t : t + 1], fm, dm, op=mybir.AluOpType.min
            )
            # Flush finished columns early so the final DMA only waits for
            # the last group.
            if t % 4 == 3:
                nc.sync.dma_start(
                    out=out[:, t - 3 : t + 1], in_=mins[:, t - 3 : t + 1]
                )

    nc.finalize()
    return nc


def _split2(x):
    hi = x.astype(BF16)
    lo = (x - hi.astype(np.float32)).astype(BF16)
    return hi, lo


def _split3(x):
    p = x.astype(BF16)
    r1 = x - p.astype(np.float32)
    q = r1.astype(BF16)
    r = (r1 - q.astype(np.float32)).astype(BF16)
    return p, q, r


def _prep_core(c1b, c2b, h):
    """Build vt/ut/t2c operands for one core (batch data c1b/c2b, l2 half h)."""
    c2h = c2b[h * L2H : (h + 1) * L2H]  # [L2H, 3]

    # U side (contour1, streamed): rows pair as
    #   (h1,hw)x3, (l1,hw)x3, (h1,lw)x3, (l1,lw)x3, (a1 parts, 1)x3
    s1 = np.sum(c1b * c1b, axis=-1, dtype=np.float32)
    sum1 = np.sum(c1b, axis=-1, dtype=np.float32)
    a1 = s1 + 2.0 * EPS * sum1
    h1, l1 = _split2(c1b.T)  # [3, L1] each
    p, q, r = _split3(a1[None, :])  # [1, L1] each
    urows = np.concatenate([h1, l1, h1, l1, p, q, r], axis=0)  # [15, L1]

    w = -2.0 * c2h.T  # [3, L2H]
    hw, lw = _split2(w)
    ones = np.ones((3, L2H), dtype=BF16)
    vrows = np.concatenate([hw, hw, lw, lw, ones], axis=0)  # [15, L2H]

    vt = np.zeros((128, L2H), dtype=BF16)
    ut = np.zeros((128, NROUND * CHUNK), dtype=BF16)
    for j in range(4):
        vt[32 * j : 32 * j + K, :] = vrows
        for r_ in range(NROUND):
            c = 4 * r_ + j
            ut[32 * j : 32 * j + K, r_ * CHUNK : (r_ + 1) * CHUNK] = urows[
                :, c * CHUNK : (c + 1) * CHUNK
            ]

    s2 = np.sum(c2h * c2h, axis=-1, dtype=np.float32)
    sum2 = np.sum(c2h, axis=-1, dtype=np.float32)
    t2 = (s2 - 2.0 * EPS * sum2 + D * EPS * EPS).astype(np.float32)
    t2c = t2.reshape(NT, 128).T.copy()  # [128, NT]; [p, t] -> l2 = t*128+p
    return vt, ut, t2c


_NC_CACHE = []


def _get_nc():
    if not _NC_CACHE:
        _NC_CACHE.append(_build_nc())
    return _NC_CACHE[0]


def kernel(contour1, contour2, resolution):
    c1 = np.asarray(contour1, dtype=np.float32)
    c2 = np.asarray(contour2, dtype=np.float32)
    res = float(np.asarray(resolution).reshape(-1)[0])

    in_maps = []
    for core in range(NCORES):
        b, h = core // 2, core % 2
        vt, ut, t2c = _prep_core(c1[b], c2[b], h)
        in_maps.append({"vt": vt, "ut": ut, "t2c": t2c})

    nc = _get_nc()
    results = bass_utils.run_bass_kernel_spmd(
        nc, in_maps, core_ids=list(range(NCORES))
    ).results

    out = np.empty((B,), dtype=np.float32)
    for b in range(B):
        halves = []
        for h in range(2):
            r = results[2 * b + h]["out"]  # [128, NT]; [p, t] -> l2 = t*128+p
            d2 = r.T.reshape(L2H)  # min dist2 per l2 point (t2 already added)
            halves.append(np.sqrt(np.maximum(d2, 0.0)))
        min_dist = np.concatenate(halves)
        out[b] = np.float32(np.mean(min_dist * res))
    return out
